# revision 18
# baseline (speedup 1.0000x reference)
"""Trainium2 Bass kernel for DegradationRectifyNet block (CSWin-style window
attention + LePE depthwise conv + code-conditioned LN/MLP).

Data-parallel over batch B=8 across 8 NeuronCores; one image per core.
On-chip everything is channel-major (C on partitions, tokens on free dim).
Compute dtype is bf16 (validated ~2.6e-3 max-rel vs the 2e-2 gate); PSUM
accumulation stays fp32.

Per-core pipeline:
  LN stats via all-ones PE matmul (partition reduction, broadcast-redundant)
  with Square/Sqrt on the scalar engine and a fast approximate reciprocal on
  DVE; LN affine + z-shift folded into transposed qkv weights. Window
  attention computes S^T (k on partitions) in 1-bank PSUM quarters (3-deep
  rotation) so exp of chunk c overlaps the matmul of chunk c+1; softmax
  denominators come from a ones-column appended to the v stationary; the
  per-query division is done in a transposed frame via bf16 PE transposes.
  LePE depthwise 3x3 runs as 9 block-diagonal PE matmuls over zero-padded
  window images (branch-1 windows stored transposed so both branches share
  geometry). Proj and the MLP are bf16 matmuls with biases/residuals fused
  into scalar_tensor_tensor epilogues; the MLP phase is split into a stats
  pass (sqrt table) and a gelu pass (gelu table) to avoid act-table thrash.

PE quadrant constraints (operand partition bases must be 32-aligned) are
handled by: (a) computing QK^T with 64-row contractions against k copies that
have the other heads zeroed (kst slabs, double-buffered across windows), and
(b) padding AV output stripes to 32 partitions with explicit zero columns in
the stationary.
"""

import numpy as np

import concourse.bass as bass
import concourse.bacc as bacc
import concourse.tile as tile
from concourse import mybir
from concourse import bass_utils
from concourse.masks import make_identity
from contextlib import ExitStack

F32 = mybir.dt.float32
BF = mybir.dt.bfloat16
AF = mybir.ActivationFunctionType
OP = mybir.AluOpType

B = 8
C = 128          # channels
H = W = 64
L = H * W        # 4096 tokens
SS = 8           # split size
CB = C // 2      # branch channels = 64
NH = 4           # heads per branch
D = CB // NH     # head dim = 16
HID = 4 * C      # 512
CHZ = 512
NT = 8           # token tiles
TT = 512         # tokens per tile
NWIN = 8         # windows per branch
EPS = 1e-5

INPUT_NAMES = [
    "x", "z", "ln1_g", "ln1_b", "ln2_g", "ln2_b", "Wz1", "Wz2", "Wqkv",
    "Wproj", "bproj", "lepe_w0", "lepe_b0", "lepe_w1", "lepe_b1",
    "W1", "b1", "W2", "b2",
]
INPUT_SHAPES = {
    "x": [C, L], "z": [CHZ],
    "ln1_g": [C], "ln1_b": [C], "ln2_g": [C], "ln2_b": [C],
    "Wz1": [C, CHZ], "Wz2": [C, CHZ], "Wqkv": [3 * C, C],
    "Wproj": [C, C], "bproj": [C],
    "lepe_w0": [CB, 1, 3, 3], "lepe_b0": [CB],
    "lepe_w1": [CB, 1, 3, 3], "lepe_b1": [CB],
    "W1": [HID, C], "b1": [HID], "W2": [C, HID], "b2": [C],
}


def emit(ctx: ExitStack, tc: tile.TileContext, io: dict):
    nc = tc.nc

    consts = ctx.enter_context(tc.tile_pool(name="consts", bufs=1))
    wpool = ctx.enter_context(tc.tile_pool(name="wpool", bufs=1))
    big = ctx.enter_context(tc.tile_pool(name="big", bufs=1))
    stat = ctx.enter_context(tc.tile_pool(name="stat", bufs=2))
    padp = ctx.enter_context(tc.tile_pool(name="padp", bufs=2))
    expp = ctx.enter_context(tc.tile_pool(name="expp", bufs=2))
    v4p = ctx.enter_context(tc.tile_pool(name="v4p", bufs=2))
    epip = ctx.enter_context(tc.tile_pool(name="epip", bufs=2))
    kstp = ctx.enter_context(tc.tile_pool(name="kstp", bufs=1))
    gelp = ctx.enter_context(tc.tile_pool(name="gelp", bufs=2))
    outp = ctx.enter_context(tc.tile_pool(name="outp", bufs=2))
    ln3p = ctx.enter_context(tc.tile_pool(name="ln3p", bufs=1))

    # long-lived PSUM: "mm" (2 banks) for f32 matmul outs, "tp" (2 banks)
    # for bf16 transpose outs. Phase-scoped pools below add <= 4 banks.
    psum = ctx.enter_context(tc.tile_pool(name="psum", bufs=1, space="PSUM"))

    def ps_mm(shape, name):
        return psum.tile(shape, F32, tag="mm", bufs=2, name=name)

    def ps_tp(shape, name):
        return psum.tile(shape, BF, tag="tp", bufs=2, name=name)

    dma = nc.sync.dma_start
    mm = nc.tensor.matmul

    # ---------------- constants + x load (phase-1 critical path first) ----
    xT = big.tile([128, L], F32, tag="xT")
    for t in range(NT):
        dma(xT[:, TT * t : TT * t + TT], io["x"][:, TT * t : TT * t + TT])
    ident = consts.tile([128, 128], F32, tag="ident")
    make_identity(nc, ident[:])
    ones_st = consts.tile([128, 128], BF, tag="ones_st")
    nc.gpsimd.memset(ones_st[:], 1.0)
    epscol = consts.tile([128, 1], F32, tag="epscol")
    nc.gpsimd.memset(epscol[:], EPS)

    def col(name):
        t = consts.tile([128, 1], F32, tag="col_" + name)
        dma(t[:], io[name].unsqueeze(1))
        return t

    g1c = col("ln1_g")
    bln1 = col("ln1_b")
    g2c = col("ln2_g")
    bln2 = col("ln2_b")
    bprojc = col("bproj")
    b2c = col("b2")

    b1cols = consts.tile([128, 4], F32, tag="b1cols")
    for h in range(4):
        dma(b1cols[:, h : h + 1], io["b1"][128 * h : 128 * h + 128].unsqueeze(1))
    zcols = consts.tile([128, 4], F32, tag="zcols")
    for k in range(4):
        dma(zcols[:, k : k + 1], io["z"][128 * k : 128 * k + 128].unsqueeze(1))

    # ---------------- weight transposes ----------------
    def load_transposed(dst_ap, src_ap, scale_col=None, copy_dst=None):
        wt = wpool.tile([128, 128], F32, tag="wtmp", bufs=4)
        dma(wt[:], src_ap)
        pt = ps_mm([128, 128], "ptw")
        nc.tensor.transpose(pt[:], wt[:], ident[:])
        if scale_col is not None:
            nc.vector.tensor_scalar_mul(dst_ap, pt[:], scale_col)
        else:
            nc.vector.tensor_copy(dst_ap, pt[:])
        if copy_dst is not None:
            nc.vector.tensor_copy(copy_dst, pt[:])

    WqkvT = wpool.tile([128, 3 * C], F32, tag="WqkvT")
    WgT = wpool.tile([128, 3 * C], BF, tag="WgT")
    for j in range(3):
        load_transposed(
            WgT[:, 128 * j : 128 * j + 128],
            io["Wqkv"][128 * j : 128 * j + 128, :],
            scale_col=g1c[:],
            copy_dst=WqkvT[:, 128 * j : 128 * j + 128],
        )

    Wz1T = wpool.tile([128, CHZ], F32, tag="Wz1T")
    for k in range(4):
        load_transposed(
            Wz1T[:, 128 * k : 128 * k + 128], io["Wz1"][:, 128 * k : 128 * k + 128]
        )

    # ---------------- z-conditioned bias columns ----------------
    def zbias(WzT, blnc, tag):
        zp = ps_mm([128, 1], "zp")
        for k in range(4):
            mm(
                zp[:], WzT[:, 128 * k : 128 * k + 128], zcols[:, k : k + 1],
                start=(k == 0), stop=(k == 3),
            )
        bz = consts.tile([128, 1], F32, tag=tag)
        nc.vector.tensor_add(bz[:], zp[:], blnc[:])
        return bz

    bz1 = zbias(Wz1T, bln1, "bz1")

    w0cols = consts.tile([128, 3], F32, tag="w0cols")
    for j in range(3):
        wp = ps_mm([128, 1], "wp")
        mm(wp[:], WqkvT[:, 128 * j : 128 * j + 128], bz1[:], start=True, stop=True)
        nc.vector.tensor_copy(w0cols[:, j : j + 1], wp[:])

    # ---------------- big activation buffers ----------------
    qT = big.tile([128, L], BF, tag="qT")
    kT = big.tile([128, L], BF, tag="kT")
    vT = big.tile([128, L], BF, tag="vT")
    kst4 = [
        [
            kstp.tile([128, TT], BF, tag=f"kst{i}_{p}", name=f"kst{i}_{p}")
            for p in range(2)
        ]
        for i in range(NH)
    ]
    cat = big.tile([128, L], BF, tag="cat")
    xf2 = big.tile([128, L], F32, tag="xf2")

    # ---------------- layernorm over channel (partition) dim ----------------
    # returns hn = (x - mean)/sqrt(var+eps) in bf16; keep_rxc also returns
    # (r, xc) tiles for the split phase-3 pipeline.
    def layernorm_tile(src, t, half_psum):
        sl = slice(TT * t, TT * t + TT)
        xt = src[:, sl]
        xb = stat.tile([128, TT], BF, tag="xb")
        nc.gpsimd.tensor_copy(xb[:], xt)
        xsq = stat.tile([128, TT], BF, tag="xsq")
        nc.scalar.activation(xsq[:], xb[:], AF.Square)
        s12 = half_psum()
        s1 = s12[:, 0:TT]
        s2 = s12[:, TT : 2 * TT]
        mm(s1, ones_st[:], xb[:], start=True, stop=True)
        mm(s2, ones_st[:], xsq[:], start=True, stop=True)
        msq = stat.tile([128, TT], F32, tag="msq")
        nc.scalar.activation(msq[:], s1, AF.Square, scale=1.0 / C)
        var = stat.tile([128, TT], F32, tag="var")
        nc.vector.scalar_tensor_tensor(
            var[:], s2, 1.0 / C, msq[:], op0=OP.mult, op1=OP.subtract
        )
        sd = stat.tile([128, TT], F32, tag="sd")
        nc.scalar.activation(sd[:], var[:], AF.Sqrt, bias=epscol[:], scale=1.0)
        r = stat.tile([128, TT], F32, tag="r")
        nc.vector.reciprocal_approx_fast(out=r[:], in_=sd[:])
        xc = stat.tile([128, TT], BF, tag="xc")
        nc.vector.scalar_tensor_tensor(
            xc[:], s1, -1.0 / C, xt, op0=OP.mult, op1=OP.add
        )
        return r, xc

    # ---------------- phase 1: LN1 + qkv ----------------
    with tc.tile_pool(name="ps1", bufs=1, space="PSUM") as ps1:
        for t in range(NT):
            sl = slice(TT * t, TT * t + TT)
            r, xc = layernorm_tile(
                xT, t,
                lambda: ps1.tile([128, 2 * TT], F32, tag="half", bufs=2,
                                 name="s12"),
            )
            hn = stat.tile([128, TT], BF, tag="hn")
            nc.vector.tensor_mul(hn[:], xc[:], r[:])
            for j, dstT in enumerate((qT, kT, vT)):
                qp = ps_mm([128, TT], "qp")
                mm(
                    qp[:], WgT[:, 128 * j : 128 * j + 128], hn[:],
                    start=True, stop=True,
                )
                if j == 2:
                    nc.vector.tensor_scalar_add(
                        dstT[:, sl], qp[:], w0cols[:, j : j + 1]
                    )
                else:
                    nc.scalar.activation(
                        dstT[:, sl], qp[:], AF.Identity,
                        bias=w0cols[:, j : j + 1], scale=1.0,
                    )

    # ---------------- attention constants (after phase-1 emission) ----------
    identB = consts.tile([128, 128], BF, tag="identB")
    nc.vector.tensor_copy(identB[:], ident[:])
    # 64x64 identity living at partitions 64:128 (for branch-1 v transposes)
    ident2f = consts.tile([128, 64], F32, tag="ident2f")
    nc.gpsimd.memset(ident2f[:], 0.0)
    nc.gpsimd.affine_select(
        out=ident2f[:], in_=ident2f[:], compare_op=OP.not_equal, fill=1.0,
        base=-64, pattern=[[-1, 64]], channel_multiplier=1,
    )
    ident2 = consts.tile([128, 64], BF, tag="ident2")
    nc.vector.tensor_copy(ident2[:], ident2f[:])
    lepebc = consts.tile([128, 1], F32, tag="lepebc")
    dma(lepebc[0:CB, :], io["lepe_b0"].unsqueeze(1))
    dma(lepebc[CB:128, :], io["lepe_b1"].unsqueeze(1))

    # conv tap weights: rows 0:64 branch-0 taps (a,b); rows 64:128 branch-1
    # taps transposed (branch-1 window images are stored transposed)
    wcomb = consts.tile([128, 9], F32, tag="wcomb")
    dma(wcomb[0:CB, :], io["lepe_w0"].rearrange("c o a b -> c (o a b)"))
    wstage = consts.tile([64, 9], F32, tag="wstage")
    dma(wstage[:], io["lepe_w1"].rearrange("c o a b -> c (o a b)"))
    nc.gpsimd.tensor_copy(
        wcomb[CB:128, :].rearrange("c (b a) -> c b a", b=3),
        wstage[:].rearrange("c (a b) -> c a b", a=3).transpose([0, 2, 1]),
    )
    # v4 template: 32-wide head slots, col 16 = 1.0 (denominator column)
    v4t = consts.tile([128, TT], BF, tag="v4t")
    nc.gpsimd.memset(v4t[:], 0.0)
    nc.vector.memset(
        v4t[:].rearrange("p (s w) -> p s w", s=16)[:, :, 16:17], 1.0
    )
    # QK^T stationary slabs: slab (h, parity) keeps only head h's 16 rows
    # live per branch (rows 64*br+16*h), rest stay zero, so the 64-row
    # contraction reads start at partition 0/64. Two parity buffers let
    # window j+1's DMAs land while j is computing.
    for i in range(NH):
        for p in range(2):
            nc.gpsimd.memset(kst4[i][p][:], 0.0)

    # ---------------- remaining weight setup (overlaps attention start) ----
    Wz2T = wpool.tile([128, CHZ], F32, tag="Wz2T")
    for k in range(4):
        load_transposed(
            Wz2T[:, 128 * k : 128 * k + 128], io["Wz2"][:, 128 * k : 128 * k + 128]
        )
    bz2 = zbias(Wz2T, bln2, "bz2")

    W1T = wpool.tile([128, HID], F32, tag="W1T")
    W1gT = wpool.tile([128, HID], BF, tag="W1gT")
    for h in range(4):
        load_transposed(
            W1gT[:, 128 * h : 128 * h + 128],
            io["W1"][128 * h : 128 * h + 128, :],
            scale_col=g2c[:],
            copy_dst=W1T[:, 128 * h : 128 * h + 128],
        )
    W2T = wpool.tile([128, HID], BF, tag="W2T")
    for h in range(4):
        load_transposed(
            W2T[:, 128 * h : 128 * h + 128], io["W2"][:, 128 * h : 128 * h + 128]
        )
    WpT = wpool.tile([128, C], BF, tag="WpT")
    load_transposed(WpT[:], io["Wproj"][:, :])

    gbcols = consts.tile([128, 4], F32, tag="gbcols")
    for h in range(4):
        wp = ps_mm([128, 1], "wp")
        mm(wp[:], W1T[:, 128 * h : 128 * h + 128], bz2[:], start=True, stop=True)
        nc.vector.tensor_add(gbcols[:, h : h + 1], wp[:], b1cols[:, h : h + 1])

    # window access patterns -------------------------------------------------
    # branch 0: vertical strip window j = cols [8j,8j+8); raster (h, w)
    # branch 1: horizontal strip window j, stored transposed; raster (w, h)
    def win_ap(src, br, j, p0, psz):
        a3 = src[p0 : p0 + psz, :].rearrange("c (h w) -> c h w", h=H)
        if br == 0:
            return a3[:, :, SS * j : SS * j + SS]
        return a3[:, SS * j : SS * j + SS, :].transpose([0, 2, 1])

    # ---------------- attention + lepe ------------------------------------
    # Branch-0 windows first (phase A, with the LePE conv for both branches),
    # then branch-1 windows (phase B). After branch-1 window j, cat tile j is
    # complete, so proj and the LN2 stats for tile j run overlapped with the
    # remaining attention. Sqrt/gelu-table work stays in a short tail.
    lepe8 = big.tile([128, L], BF, tag="lepe8")  # br1 lepe halves (rows 64:)
    var8 = []
    xc8 = []

    with tc.tile_pool(name="ps2", bufs=1, space="PSUM") as ps2:

        def build_v4(j, br, vst):
            # v': token-major v (via PE transpose), 32-wide head slots:
            # cols [0:16) v, col 16 ones (denominator), cols [17:32) zero
            p0 = CB * br
            vps = ps_tp([128, 256], "vps")
            idv = identB[0:CB, 0:CB] if br == 0 else ident2[CB:128, :]
            for c in range(4):
                mm(
                    vps[:, 64 * c : 64 * c + 64],
                    vst[p0 : p0 + CB, 128 * c : 128 * c + 128],
                    idv,
                    is_transpose=True,
                    start=(c == 0), stop=(c == 3),
                )
            v4 = v4p.tile([128, TT], BF, tag="v4")
            nc.vector.tensor_copy(v4[:], v4t[:])
            v4v = v4[:].rearrange("p (c h s) -> p c h s", c=4, h=4)
            nc.vector.tensor_copy(
                v4v[:, :, :, 0:16],
                vps[:].rearrange("p (c h d) -> p c h d", c=4, h=4),
            )
            return v4

        def branch_attn(j, br, qw, v4, lepe_view):
            par = j % 2
            p0 = CB * br
            A = epip.tile([128, TT], BF, tag="A")
            for h in range(NH):
                kst = kst4[h][par]
                es = expp.tile([128, 4 * TT], BF, tag="es")
                # 2-bank score halves, 2-deep rotation: exp of half a
                # overlaps the matmuls of half a+1
                for a in range(2):
                    sp = ps2.tile([128, 2 * TT], F32, tag="sp", bufs=2,
                                  name="sp")
                    for cc in range(2):
                        c = 2 * a + cc
                        mm(
                            sp[:, TT * cc : TT * cc + TT],
                            kst[p0 : p0 + CB, 128 * c : 128 * c + 128],
                            qw,
                            start=True, stop=True,
                        )
                    nc.scalar.activation(
                        es[:, 2 * TT * a : 2 * TT * a + 2 * TT], sp[:],
                        AF.Exp, scale=float(D) ** -0.5,
                    )
                avh = ps_mm([32, TT], "avh")
                for c in range(4):
                    mm(
                        avh[:],
                        v4[:, 128 * c + 32 * h : 128 * c + 32 * h + 32],
                        es[:, TT * c : TT * c + TT],
                        start=(c == 0), stop=(c == 3),
                    )
                nc.vector.tensor_copy(A[32 * h : 32 * h + 32, :], avh[:])

            # epilogue: transpose -> divide by denominators -> transpose
            Tp = ps_tp([128, TT], "Tp")
            for c in range(4):
                mm(
                    Tp[:, 128 * c : 128 * c + 128],
                    A[:, 128 * c : 128 * c + 128],
                    identB[:],
                    is_transpose=True,
                    start=(c == 0), stop=(c == 3),
                )
            Tv = Tp.rearrange("p (c h s) -> p c h s", c=4, h=4)
            dcol = epip.tile([128, 16], F32, tag="dcol")
            nc.vector.tensor_copy(
                dcol[:].rearrange("p (c h) -> p c h", c=4), Tv[:, :, :, 16]
            )
            R = epip.tile([128, 16], F32, tag="R")
            nc.vector.reciprocal_approx_fast(out=R[:], in_=dcol[:])
            Rv = R[:].rearrange("p (c h) -> p c h", c=4)
            E = epip.tile([128, 256], BF, tag="E")
            Ev = E[:].rearrange("p (c h d) -> p c h d", c=4, h=4)
            nc.vector.tensor_mul(
                Ev[:, :, :, :],
                Tv[:, :, :, 0:16],
                Rv[:, :, :].unsqueeze(3).broadcast_to((128, 4, 4, 16)),
            )
            Ot = ps_tp([CB, TT], "Ot")
            for c in range(4):
                mm(
                    Ot[:, 128 * c : 128 * c + 128],
                    E[:, 64 * c : 64 * c + 64],
                    identB[:],
                    is_transpose=True,
                    start=(c == 0), stop=(c == 3),
                )
            # un-window: attention + lepe(+bias) into cat rows [64*br,+64)
            nc.vector.scalar_tensor_tensor(
                win_ap(cat, br, j, p0, CB),
                Ot[:].rearrange("c (h w) -> c h w", h=H),
                lepebc[p0 : p0 + CB, :],
                lepe_view,
                op0=OP.add, op1=OP.add,
            )

        # ---- phase A: branch-0 windows + LePE conv for both branches ----
        for j in range(NWIN):
            par = j % 2
            vst = v4p.tile([128, TT], BF, tag="vst")
            dma(
                vst[0:CB, :].rearrange("c (a b) -> c a b", a=64),
                win_ap(vT, 0, j, 0, CB),
            )
            for h in range(NH):
                kst = kst4[h][par]
                dma(
                    kst[D * h : D * h + D, :].rearrange("c (a b) -> c a b",
                                                        a=64),
                    win_ap(kT, 0, j, D * h, D),
                )
            # contiguous q-window tile (strided bf16 views are PE-fetch
            # bound): br0 gather via DMA (prefetchable), br1 via DVE
            qcw = padp.tile([128, TT], BF, tag="qcw")
            qcw3 = qcw[:].rearrange("c (a b) -> c a b", a=64)
            dma(qcw3[0:CB], win_ap(qT, 0, j, 0, CB))
            nc.vector.tensor_copy(qcw3[CB:128], win_ap(qT, 1, j, CB, CB))

            # LePE: zero-padded q window images (66 x 10), branches stacked;
            # 9 taps split DVE(5)/GPSIMD(4) with two accumulators
            pad = padp.tile([128, 660], BF, tag="pad")
            nc.gpsimd.memset(pad[:], 0.0)
            pad3 = pad[:].rearrange("c (h w) -> c h w", h=66)
            nc.gpsimd.tensor_copy(pad3[:, 1:65, 1:9], qcw3)
            lepe = padp.tile([128, TT], BF, tag="lepe")
            lepe3 = lepe[:].rearrange("c (h w) -> c h w", h=64)
            taps = [(a, b) for a in (-1, 0, 1) for b in (-1, 0, 1)]
            for idx, (a, b) in enumerate(taps):
                src = pad3[:, 1 + a : 65 + a, 1 + b : 9 + b]
                wc = wcomb[:, 3 * (a + 1) + (b + 1) : 3 * (a + 1) + (b + 2)]
                if idx == 0:
                    nc.vector.tensor_scalar_mul(lepe3, src, wc)
                else:
                    nc.vector.scalar_tensor_tensor(
                        lepe3, src, wc, lepe3, op0=OP.mult, op1=OP.add
                    )
            nc.vector.tensor_copy(
                lepe8[CB:128, TT * j : TT * j + TT], lepe[CB:128, :]
            )
            v4 = build_v4(j, 0, vst)
            branch_attn(
                j, 0, qcw[0:CB, :], v4,
                lepe[0:CB, :].rearrange("c (h w) -> c h w", h=H),
            )

        # ---- phase B: branch-1 windows; proj + LN2 stats overlap --------
        for j in range(NWIN):
            par = j % 2
            vst = v4p.tile([128, TT], BF, tag="vst")
            dma(vst[CB:128, :], vT[CB:128, TT * j : TT * j + TT])
            for h in range(NH):
                kst = kst4[h][par]
                dma(
                    kst[CB + D * h : CB + D * h + D, :],
                    kT[CB + D * h : CB + D * h + D, TT * j : TT * j + TT],
                )
            qcw = padp.tile([128, TT], BF, tag="qcw")
            qcw3 = qcw[:].rearrange("c (a b) -> c a b", a=64)
            nc.vector.tensor_copy(qcw3[CB:128], win_ap(qT, 1, j, CB, CB))
            v4 = build_v4(j, 1, vst)
            branch_attn(
                j, 1, qcw[CB:128, :], v4,
                lepe8[CB:128, TT * j : TT * j + TT].rearrange(
                    "c (h w) -> c h w", h=H
                ),
            )
            # proj + residual for tile j (cat tile j is now complete)
            sl = slice(TT * j, TT * j + TT)
            ap_ = ps_mm([128, TT], "ap_")
            mm(ap_[:], WpT[:], cat[:, sl], start=True, stop=True)
            nc.vector.scalar_tensor_tensor(
                xf2[:, sl], ap_[:], bprojc[:], xT[:, sl],
                op0=OP.add, op1=OP.add,
            )
            # eager LN2 stats for tile j (Square lives in every act table;
            # the Sqrt stays in the tail to avoid exp-table thrash)
            xb = stat.tile([128, TT], BF, tag="xb")
            nc.gpsimd.tensor_copy(xb[:], xf2[:, sl])
            xsq = stat.tile([128, TT], BF, tag="xsq")
            nc.gpsimd.tensor_mul(xsq[:], xb[:], xb[:])
            s1p = psum.tile([128, TT], F32, tag="tp", bufs=2, name="s1p")
            mm(s1p[:], ones_st[:], xb[:], start=True, stop=True)
            s2p = psum.tile([128, TT], F32, tag="tp", bufs=2, name="s2p")
            mm(s2p[:], ones_st[:], xsq[:], start=True, stop=True)
            msq = stat.tile([128, TT], F32, tag="msq")
            nc.scalar.activation(msq[:], s1p[:], AF.Square, scale=1.0 / C)
            var = ln3p.tile([128, TT], F32, tag=f"var_{j}", name=f"var_{j}")
            nc.vector.scalar_tensor_tensor(
                var[:], s2p[:], 1.0 / C, msq[:], op0=OP.mult, op1=OP.subtract
            )
            xc = ln3p.tile([128, TT], BF, tag=f"xc_{j}", name=f"xc_{j}")
            nc.vector.scalar_tensor_tensor(
                xc[:], s1p[:], -1.0 / C, xf2[:, sl], op0=OP.mult, op1=OP.add
            )
            var8.append(var)
            xc8.append(xc)

    # ---------------- tail: rsqrt pass (sqrt table) then MLP (gelu) --------
    hns = []
    for t in range(NT):
        sd = stat.tile([128, TT], F32, tag="sd")
        nc.scalar.activation(
            sd[:], var8[t][:], AF.Sqrt, bias=epscol[:], scale=1.0
        )
        r = stat.tile([128, TT], F32, tag="r")
        nc.vector.reciprocal_approx_fast(out=r[:], in_=sd[:])
        hn = ln3p.tile([128, TT], BF, tag=f"hn2_{t}", name=f"hn2_{t}")
        nc.vector.tensor_mul(hn[:], xc8[t][:], r[:])
        hns.append(hn)

    with tc.tile_pool(name="ps3b", bufs=1, space="PSUM") as ps3b:
        for t in range(NT):
            sl = slice(TT * t, TT * t + TT)
            hn = hns[t]
            gel = gelp.tile([128, 4 * TT], BF, tag="gel")
            for hh in range(4):
                hp = ps3b.tile([128, TT], F32, tag="hp", bufs=2, name="hp")
                mm(
                    hp[:],
                    W1gT[:, 128 * hh : 128 * hh + 128],
                    hn[:],
                    start=True, stop=True,
                )
                nc.scalar.activation(
                    gel[:, TT * hh : TT * hh + TT],
                    hp[:],
                    AF.Gelu,
                    bias=gbcols[:, hh : hh + 1],
                    scale=1.0,
                )
            o2 = ps_mm([128, TT], "o2")
            for hh in range(4):
                mm(
                    o2[:],
                    W2T[:, 128 * hh : 128 * hh + 128],
                    gel[:, TT * hh : TT * hh + TT],
                    start=(hh == 0), stop=(hh == 3),
                )
            ot = outp.tile([128, TT], F32, tag="ot")
            nc.vector.scalar_tensor_tensor(
                ot[:], o2[:], b2c[:], xf2[:, sl], op0=OP.add, op1=OP.add
            )
            dma(io["out"][:, sl], ot[:])


_NC_CACHE = {}


def build_nc():
    key = "nc"
    if key in _NC_CACHE:
        return _NC_CACHE[key]
    nc = bacc.Bacc("TRN2", target_bir_lowering=False, debug=False)
    io = {}
    for name in INPUT_NAMES:
        io[name] = nc.dram_tensor(
            name, INPUT_SHAPES[name], F32, kind="ExternalInput"
        ).ap()
    io["out"] = nc.dram_tensor("out", [C, L], F32, kind="ExternalOutput").ap()
    with tile.TileContext(nc) as tc:
        with ExitStack() as ctx:
            emit(ctx, tc, io)
    nc.compile()
    _NC_CACHE[key] = nc
    return nc


def make_in_maps(inputs):
    in_maps = []
    for b in range(B):
        m = {
            "x": np.ascontiguousarray(
                inputs["x"][b].reshape(C, L).astype(np.float32)
            ),
            "z": np.ascontiguousarray(inputs["z"][b].astype(np.float32)),
        }
        for name in INPUT_NAMES:
            if name in ("x", "z"):
                continue
            m[name] = np.ascontiguousarray(np.asarray(inputs[name], np.float32))
        in_maps.append(m)
    return in_maps


def kernel(**inputs):
    nc = build_nc()
    in_maps = make_in_maps(inputs)
    res = bass_utils.run_bass_kernel_spmd(nc, in_maps, list(range(B)))
    out = np.stack([res.results[b]["out"].reshape(C, H, W) for b in range(B)])
    return out.astype(np.float32)


if __name__ == "__main__":
    # CoreSim numerics check of core 0 against the reference (dev only).
    import sys

    sys.path.insert(0, "/root/problem")
    import reference

    from concourse.bass_interp import CoreSim

    # CoreSim has no Gelu; patch it (HW has a native erf-gelu table).
    import scipy.special
    from concourse import bass_interp

    _orig_act = bass_interp.InstructionExecutor.visit_InstActivation

    def _patched_act(self, instruction, *, reg_snapshot=None):
        if instruction.func == mybir.ActivationFunctionType.Gelu:
            instruction.func = mybir.ActivationFunctionType.Identity
            try:
                _orig_act(self, instruction, reg_snapshot=reg_snapshot)
            finally:
                instruction.func = mybir.ActivationFunctionType.Gelu
            ov = self.view_ap(
                instruction.outs[0],
                bass_interp.Direction.WRITE,
                instruction,
                reg_snapshot=reg_snapshot,
            )
            x = ov.astype(np.float64)
            ov[:] = (
                x * 0.5 * (1.0 + scipy.special.erf(x / np.sqrt(2.0)))
            ).astype(ov.dtype)
            return
        return _orig_act(self, instruction, reg_snapshot=reg_snapshot)

    bass_interp.InstructionExecutor.visit_InstActivation = _patched_act

    inputs = {k: np.asarray(v) for k, v in reference.setup_inputs().items()}
    expected = np.asarray(reference.reference(**inputs))

    nc = build_nc()
    print("built+compiled", flush=True)
    sim = CoreSim(nc, require_finite=True, require_nnan=True)
    m = make_in_maps(inputs)[0]
    for k, v in m.items():
        sim.tensor(k)[:] = v
    sim.simulate(check_with_hw=False)
    got = sim.tensor("out").reshape(C, H, W)
    exp0 = expected[0]
    err = np.abs(got - exp0)
    denom = np.abs(exp0).max()
    print("absmax err:", err.max(), "rel:", err.max() / denom)
    print(
        "rms rel:",
        np.sqrt(((got - exp0) ** 2).mean()) / np.sqrt((exp0**2).mean()),
    )


# revision 22
# speedup vs baseline: 1.0281x; 1.0281x over previous
"""Trainium2 Bass kernel for DegradationRectifyNet block (CSWin-style window
attention + LePE depthwise conv + code-conditioned LN/MLP).

Data-parallel over batch B=8 across 8 NeuronCores; one image per core.
On-chip everything is channel-major (C on partitions, tokens on free dim).
Compute dtype is bf16 (validated ~2.6e-3 max-rel vs the 2e-2 gate); PSUM
accumulation stays fp32.

Per-core pipeline:
  LN stats via all-ones PE matmul (partition reduction, broadcast-redundant)
  with Square/Sqrt on the scalar engine and a fast approximate reciprocal on
  DVE; LN affine + z-shift folded into transposed qkv weights. Window
  attention computes S^T (k on partitions) in 1-bank PSUM quarters (3-deep
  rotation) so exp of chunk c overlaps the matmul of chunk c+1; softmax
  denominators come from a ones-column appended to the v stationary; the
  per-query division is done in a transposed frame via bf16 PE transposes.
  LePE depthwise 3x3 runs as 9 block-diagonal PE matmuls over zero-padded
  window images (branch-1 windows stored transposed so both branches share
  geometry). Proj and the MLP are bf16 matmuls with biases/residuals fused
  into scalar_tensor_tensor epilogues; the MLP phase is split into a stats
  pass (sqrt table) and a gelu pass (gelu table) to avoid act-table thrash.

PE quadrant constraints (operand partition bases must be 32-aligned) are
handled by: (a) computing QK^T with 64-row contractions against k copies that
have the other heads zeroed (kst slabs, double-buffered across windows), and
(b) padding AV output stripes to 32 partitions with explicit zero columns in
the stationary.
"""

import numpy as np

import concourse.bass as bass
import concourse.bacc as bacc
import concourse.tile as tile
from concourse import mybir
from concourse import bass_utils
from concourse.masks import make_identity
from contextlib import ExitStack

F32 = mybir.dt.float32
BF = mybir.dt.bfloat16
AF = mybir.ActivationFunctionType
OP = mybir.AluOpType

B = 8
C = 128          # channels
H = W = 64
L = H * W        # 4096 tokens
SS = 8           # split size
CB = C // 2      # branch channels = 64
NH = 4           # heads per branch
D = CB // NH     # head dim = 16
HID = 4 * C      # 512
CHZ = 512
NT = 8           # token tiles
TT = 512         # tokens per tile
NWIN = 8         # windows per branch
EPS = 1e-5

INPUT_NAMES = [
    "x", "z", "ln1_g", "ln1_b", "ln2_g", "ln2_b", "Wz1", "Wz2", "Wqkv",
    "Wproj", "bproj", "lepe_w0", "lepe_b0", "lepe_w1", "lepe_b1",
    "W1", "b1", "W2", "b2",
]
INPUT_SHAPES = {
    "x": [C, L], "z": [CHZ],
    "ln1_g": [C], "ln1_b": [C], "ln2_g": [C], "ln2_b": [C],
    "Wz1": [C, CHZ], "Wz2": [C, CHZ], "Wqkv": [3 * C, C],
    "Wproj": [C, C], "bproj": [C],
    "lepe_w0": [CB, 1, 3, 3], "lepe_b0": [CB],
    "lepe_w1": [CB, 1, 3, 3], "lepe_b1": [CB],
    "W1": [HID, C], "b1": [HID], "W2": [C, HID], "b2": [C],
}


def emit(ctx: ExitStack, tc: tile.TileContext, io: dict):
    nc = tc.nc

    consts = ctx.enter_context(tc.tile_pool(name="consts", bufs=1))
    wpool = ctx.enter_context(tc.tile_pool(name="wpool", bufs=1))
    big = ctx.enter_context(tc.tile_pool(name="big", bufs=1))
    stat = ctx.enter_context(tc.tile_pool(name="stat", bufs=2))
    padp = ctx.enter_context(tc.tile_pool(name="padp", bufs=2))
    expp = ctx.enter_context(tc.tile_pool(name="expp", bufs=2))
    v4p = ctx.enter_context(tc.tile_pool(name="v4p", bufs=2))
    epip = ctx.enter_context(tc.tile_pool(name="epip", bufs=2))
    kstp = ctx.enter_context(tc.tile_pool(name="kstp", bufs=1))
    gelp = ctx.enter_context(tc.tile_pool(name="gelp", bufs=2))
    outp = ctx.enter_context(tc.tile_pool(name="outp", bufs=2))
    ln3p = ctx.enter_context(tc.tile_pool(name="ln3p", bufs=1))

    # long-lived PSUM: "mm" (2 banks) for f32 matmul outs, "tp" (2 banks)
    # for bf16 transpose outs. Phase-scoped pools below add <= 4 banks.
    psum = ctx.enter_context(tc.tile_pool(name="psum", bufs=1, space="PSUM"))

    def ps_mm(shape, name):
        return psum.tile(shape, F32, tag="mm", bufs=2, name=name)

    def ps_tp(shape, name):
        return psum.tile(shape, BF, tag="tp", bufs=2, name=name)

    dma = nc.sync.dma_start
    mm = nc.tensor.matmul

    # ---------------- constants + x load (phase-1 critical path first) ----
    xT = big.tile([128, L], F32, tag="xT")
    for t in range(NT):
        dma(xT[:, TT * t : TT * t + TT], io["x"][:, TT * t : TT * t + TT])
    ident = consts.tile([128, 128], F32, tag="ident")
    make_identity(nc, ident[:])
    ones_st = consts.tile([128, 128], BF, tag="ones_st")
    nc.gpsimd.memset(ones_st[:], 1.0)
    epscol = consts.tile([128, 1], F32, tag="epscol")
    nc.gpsimd.memset(epscol[:], EPS)

    def col(name):
        t = consts.tile([128, 1], F32, tag="col_" + name)
        dma(t[:], io[name].unsqueeze(1))
        return t

    g1c = col("ln1_g")
    bln1 = col("ln1_b")
    g2c = col("ln2_g")
    bln2 = col("ln2_b")
    bprojc = col("bproj")
    b2c = col("b2")

    b1cols = consts.tile([128, 4], F32, tag="b1cols")
    for h in range(4):
        dma(b1cols[:, h : h + 1], io["b1"][128 * h : 128 * h + 128].unsqueeze(1))
    zcols = consts.tile([128, 4], F32, tag="zcols")
    for k in range(4):
        dma(zcols[:, k : k + 1], io["z"][128 * k : 128 * k + 128].unsqueeze(1))

    # ---------------- weight transposes ----------------
    def load_transposed(dst_ap, src_ap, scale_col=None, copy_dst=None):
        wt = wpool.tile([128, 128], F32, tag="wtmp", bufs=4)
        dma(wt[:], src_ap)
        pt = ps_mm([128, 128], "ptw")
        nc.tensor.transpose(pt[:], wt[:], ident[:])
        if scale_col is not None:
            nc.vector.tensor_scalar_mul(dst_ap, pt[:], scale_col)
        else:
            nc.vector.tensor_copy(dst_ap, pt[:])
        if copy_dst is not None:
            nc.vector.tensor_copy(copy_dst, pt[:])

    WqkvT = wpool.tile([128, 3 * C], F32, tag="WqkvT")
    WgT = wpool.tile([128, 3 * C], BF, tag="WgT")
    for j in range(3):
        load_transposed(
            WgT[:, 128 * j : 128 * j + 128],
            io["Wqkv"][128 * j : 128 * j + 128, :],
            scale_col=g1c[:],
            copy_dst=WqkvT[:, 128 * j : 128 * j + 128],
        )

    Wz1T = wpool.tile([128, CHZ], F32, tag="Wz1T")
    for k in range(4):
        load_transposed(
            Wz1T[:, 128 * k : 128 * k + 128], io["Wz1"][:, 128 * k : 128 * k + 128]
        )

    # ---------------- z-conditioned bias columns ----------------
    def zbias(WzT, blnc, tag):
        zp = ps_mm([128, 1], "zp")
        for k in range(4):
            mm(
                zp[:], WzT[:, 128 * k : 128 * k + 128], zcols[:, k : k + 1],
                start=(k == 0), stop=(k == 3),
            )
        bz = consts.tile([128, 1], F32, tag=tag)
        nc.vector.tensor_add(bz[:], zp[:], blnc[:])
        return bz

    bz1 = zbias(Wz1T, bln1, "bz1")

    w0cols = consts.tile([128, 3], F32, tag="w0cols")
    for j in range(3):
        wp = ps_mm([128, 1], "wp")
        mm(wp[:], WqkvT[:, 128 * j : 128 * j + 128], bz1[:], start=True, stop=True)
        nc.vector.tensor_copy(w0cols[:, j : j + 1], wp[:])

    # ---------------- big activation buffers ----------------
    qT = big.tile([128, L], BF, tag="qT")
    kT = big.tile([128, L], BF, tag="kT")
    vT = big.tile([128, L], BF, tag="vT")
    kst4 = [
        [
            kstp.tile([128, TT], BF, tag=f"kst{i}_{p}", name=f"kst{i}_{p}")
            for p in range(2)
        ]
        for i in range(NH)
    ]
    cat = big.tile([128, L], BF, tag="cat")
    xf2 = big.tile([128, L], F32, tag="xf2")

    # ---------------- layernorm over channel (partition) dim ----------------
    # returns hn = (x - mean)/sqrt(var+eps) in bf16; keep_rxc also returns
    # (r, xc) tiles for the split phase-3 pipeline.
    def layernorm_tile(src, t, half_psum):
        sl = slice(TT * t, TT * t + TT)
        xt = src[:, sl]
        xb = stat.tile([128, TT], BF, tag="xb")
        nc.gpsimd.tensor_copy(xb[:], xt)
        xsq = stat.tile([128, TT], BF, tag="xsq")
        nc.scalar.activation(xsq[:], xb[:], AF.Square)
        s12 = half_psum()
        s1 = s12[:, 0:TT]
        s2 = s12[:, TT : 2 * TT]
        mm(s1, ones_st[:], xb[:], start=True, stop=True)
        mm(s2, ones_st[:], xsq[:], start=True, stop=True)
        msq = stat.tile([128, TT], F32, tag="msq")
        nc.scalar.activation(msq[:], s1, AF.Square, scale=1.0 / C)
        var = stat.tile([128, TT], F32, tag="var")
        nc.vector.scalar_tensor_tensor(
            var[:], s2, 1.0 / C, msq[:], op0=OP.mult, op1=OP.subtract
        )
        sd = stat.tile([128, TT], F32, tag="sd")
        nc.scalar.activation(sd[:], var[:], AF.Sqrt, bias=epscol[:], scale=1.0)
        r = stat.tile([128, TT], F32, tag="r")
        nc.vector.reciprocal_approx_fast(out=r[:], in_=sd[:])
        xc = stat.tile([128, TT], BF, tag="xc")
        nc.vector.scalar_tensor_tensor(
            xc[:], s1, -1.0 / C, xt, op0=OP.mult, op1=OP.add
        )
        return r, xc

    # ---------------- phase 1: LN1 + qkv ----------------
    with tc.tile_pool(name="ps1", bufs=1, space="PSUM") as ps1:
        for t in range(NT):
            sl = slice(TT * t, TT * t + TT)
            r, xc = layernorm_tile(
                xT, t,
                lambda: ps1.tile([128, 2 * TT], F32, tag="half", bufs=2,
                                 name="s12"),
            )
            hn = stat.tile([128, TT], BF, tag="hn")
            nc.vector.tensor_mul(hn[:], xc[:], r[:])
            for j, dstT in enumerate((qT, kT, vT)):
                qp = ps_mm([128, TT], "qp")
                mm(
                    qp[:], WgT[:, 128 * j : 128 * j + 128], hn[:],
                    start=True, stop=True,
                )
                if j == 2:
                    nc.vector.tensor_scalar_add(
                        dstT[:, sl], qp[:], w0cols[:, j : j + 1]
                    )
                else:
                    nc.scalar.activation(
                        dstT[:, sl], qp[:], AF.Identity,
                        bias=w0cols[:, j : j + 1], scale=1.0,
                    )

    # ---------------- attention constants (after phase-1 emission) ----------
    identB = consts.tile([128, 128], BF, tag="identB")
    nc.vector.tensor_copy(identB[:], ident[:])
    # 64x64 identity living at partitions 64:128 (for branch-1 v transposes)
    ident2f = consts.tile([128, 64], F32, tag="ident2f")
    nc.gpsimd.memset(ident2f[:], 0.0)
    nc.gpsimd.affine_select(
        out=ident2f[:], in_=ident2f[:], compare_op=OP.not_equal, fill=1.0,
        base=-64, pattern=[[-1, 64]], channel_multiplier=1,
    )
    ident2 = consts.tile([128, 64], BF, tag="ident2")
    nc.vector.tensor_copy(ident2[:], ident2f[:])
    lepebc = consts.tile([128, 1], F32, tag="lepebc")
    dma(lepebc[0:CB, :], io["lepe_b0"].unsqueeze(1))
    dma(lepebc[CB:128, :], io["lepe_b1"].unsqueeze(1))

    # conv tap weights: rows 0:64 branch-0 taps (a,b); rows 64:128 branch-1
    # taps transposed (branch-1 window images are stored transposed)
    wcomb = consts.tile([128, 9], F32, tag="wcomb")
    dma(wcomb[0:CB, :], io["lepe_w0"].rearrange("c o a b -> c (o a b)"))
    wstage = consts.tile([64, 9], F32, tag="wstage")
    dma(wstage[:], io["lepe_w1"].rearrange("c o a b -> c (o a b)"))
    nc.gpsimd.tensor_copy(
        wcomb[CB:128, :].rearrange("c (b a) -> c b a", b=3),
        wstage[:].rearrange("c (a b) -> c a b", a=3).transpose([0, 2, 1]),
    )
    # v4 template: 32-wide head slots, col 16 = 1.0 (denominator column)
    v4t = consts.tile([128, TT], BF, tag="v4t")
    nc.gpsimd.memset(v4t[:], 0.0)
    nc.vector.memset(
        v4t[:].rearrange("p (s w) -> p s w", s=16)[:, :, 16:17], 1.0
    )
    # QK^T stationary slabs: slab (h, parity) keeps only head h's 16 rows
    # live per branch (rows 64*br+16*h), rest stay zero, so the 64-row
    # contraction reads start at partition 0/64. Two parity buffers let
    # window j+1's DMAs land while j is computing.
    for i in range(NH):
        for p in range(2):
            nc.gpsimd.memset(kst4[i][p][:], 0.0)

    # ---------------- remaining weight setup (overlaps attention start) ----
    Wz2T = wpool.tile([128, CHZ], F32, tag="Wz2T")
    for k in range(4):
        load_transposed(
            Wz2T[:, 128 * k : 128 * k + 128], io["Wz2"][:, 128 * k : 128 * k + 128]
        )
    bz2 = zbias(Wz2T, bln2, "bz2")

    W1T = wpool.tile([128, HID], F32, tag="W1T")
    W1gT = wpool.tile([128, HID], BF, tag="W1gT")
    for h in range(4):
        load_transposed(
            W1gT[:, 128 * h : 128 * h + 128],
            io["W1"][128 * h : 128 * h + 128, :],
            scale_col=g2c[:],
            copy_dst=W1T[:, 128 * h : 128 * h + 128],
        )
    W2T = wpool.tile([128, HID], BF, tag="W2T")
    for h in range(4):
        load_transposed(
            W2T[:, 128 * h : 128 * h + 128], io["W2"][:, 128 * h : 128 * h + 128]
        )
    WpT = wpool.tile([128, C], BF, tag="WpT")
    load_transposed(WpT[:], io["Wproj"][:, :])

    gbcols = consts.tile([128, 4], F32, tag="gbcols")
    for h in range(4):
        wp = ps_mm([128, 1], "wp")
        mm(wp[:], W1T[:, 128 * h : 128 * h + 128], bz2[:], start=True, stop=True)
        nc.vector.tensor_add(gbcols[:, h : h + 1], wp[:], b1cols[:, h : h + 1])

    # window access patterns -------------------------------------------------
    # branch 0: vertical strip window j = cols [8j,8j+8); raster (h, w)
    # branch 1: horizontal strip window j, stored transposed; raster (w, h)
    def win_ap(src, br, j, p0, psz):
        a3 = src[p0 : p0 + psz, :].rearrange("c (h w) -> c h w", h=H)
        if br == 0:
            return a3[:, :, SS * j : SS * j + SS]
        return a3[:, SS * j : SS * j + SS, :].transpose([0, 2, 1])

    # ---------------- attention + lepe ------------------------------------
    # Branch-0 windows first (phase A, with the LePE conv for both branches),
    # then branch-1 windows (phase B). After branch-1 window j, cat tile j is
    # complete, so proj and the LN2 stats for tile j run overlapped with the
    # remaining attention. Sqrt/gelu-table work stays in a short tail.
    lepe8 = big.tile([128, L], BF, tag="lepe8")  # br1 lepe halves (rows 64:)
    var8 = []
    xc8 = []

    with tc.tile_pool(name="ps2", bufs=1, space="PSUM") as ps2:

        def build_v4(j, br, vst):
            # v': token-major v (via PE transpose), 32-wide head slots:
            # cols [0:16) v, col 16 ones (denominator), cols [17:32) zero
            p0 = CB * br
            vps = ps_tp([128, 256], "vps")
            idv = identB[0:CB, 0:CB] if br == 0 else ident2[CB:128, :]
            for c in range(4):
                mm(
                    vps[:, 64 * c : 64 * c + 64],
                    vst[p0 : p0 + CB, 128 * c : 128 * c + 128],
                    idv,
                    is_transpose=True,
                    start=(c == 0), stop=(c == 3),
                )
            v4 = v4p.tile([128, TT], BF, tag="v4")
            nc.vector.tensor_copy(v4[:], v4t[:])
            v4v = v4[:].rearrange("p (c h s) -> p c h s", c=4, h=4)
            nc.vector.tensor_copy(
                v4v[:, :, :, 0:16],
                vps[:].rearrange("p (c h d) -> p c h d", c=4, h=4),
            )
            return v4

        def branch_attn(j, br, qw, v4, lepe_view):
            par = j % 2
            p0 = CB * br
            # all 4 heads' AV accumulate into one PSUM tile at 32-aligned
            # partition offsets (bf16 matmuls allow nonzero col tile base)
            Aps = ps_mm([128, TT], "Aps")
            A = epip.tile([128, TT], BF, tag="A")
            for h in range(NH):
                kst = kst4[h][par]
                es = expp.tile([128, 4 * TT], BF, tag="es")
                # 2-bank score halves, 2-deep rotation: exp of half a
                # overlaps the matmuls of half a+1
                for a in range(2):
                    sp = ps2.tile([128, 2 * TT], F32, tag="sp", bufs=2,
                                  name="sp")
                    for cc in range(2):
                        c = 2 * a + cc
                        mm(
                            sp[:, TT * cc : TT * cc + TT],
                            kst[p0 : p0 + CB, 128 * c : 128 * c + 128],
                            qw,
                            start=True, stop=True,
                        )
                    nc.scalar.activation(
                        es[:, 2 * TT * a : 2 * TT * a + 2 * TT], sp[:],
                        AF.Exp, scale=float(D) ** -0.5,
                    )
                for c in range(4):
                    mm(
                        Aps[32 * h : 32 * h + 32, :],
                        v4[:, 128 * c + 32 * h : 128 * c + 32 * h + 32],
                        es[:, TT * c : TT * c + TT],
                        start=(c == 0), stop=(c == 3),
                        tile_position=(0, 32 * h),
                    )
            nc.vector.tensor_copy(A[:], Aps[:])

            # epilogue: transpose -> divide by denominators -> transpose
            Tp = ps_tp([128, TT], "Tp")
            for c in range(4):
                mm(
                    Tp[:, 128 * c : 128 * c + 128],
                    A[:, 128 * c : 128 * c + 128],
                    identB[:],
                    is_transpose=True,
                    start=(c == 0), stop=(c == 3),
                )
            Tv = Tp.rearrange("p (c h s) -> p c h s", c=4, h=4)
            dcol = epip.tile([128, 16], F32, tag="dcol")
            nc.vector.tensor_copy(
                dcol[:].rearrange("p (c h) -> p c h", c=4), Tv[:, :, :, 16]
            )
            R = epip.tile([128, 16], F32, tag="R")
            nc.vector.reciprocal_approx_fast(out=R[:], in_=dcol[:])
            Rv = R[:].rearrange("p (c h) -> p c h", c=4)
            E = epip.tile([128, 256], BF, tag="E")
            Ev = E[:].rearrange("p (c h d) -> p c h d", c=4, h=4)
            nc.vector.tensor_mul(
                Ev[:, :, :, :],
                Tv[:, :, :, 0:16],
                Rv[:, :, :].unsqueeze(3).broadcast_to((128, 4, 4, 16)),
            )
            Ot = ps_tp([CB, TT], "Ot")
            for c in range(4):
                mm(
                    Ot[:, 128 * c : 128 * c + 128],
                    E[:, 64 * c : 64 * c + 64],
                    identB[:],
                    is_transpose=True,
                    start=(c == 0), stop=(c == 3),
                )
            # un-window: attention + lepe(+bias) into cat rows [64*br,+64)
            nc.vector.scalar_tensor_tensor(
                win_ap(cat, br, j, p0, CB),
                Ot[:].rearrange("c (h w) -> c h w", h=H),
                lepebc[p0 : p0 + CB, :],
                lepe_view,
                op0=OP.add, op1=OP.add,
            )

        # ---- phase A: branch-0 windows + LePE conv for both branches ----
        for j in range(NWIN):
            par = j % 2
            vst = v4p.tile([128, TT], BF, tag="vst")
            dma(
                vst[0:CB, :].rearrange("c (a b) -> c a b", a=64),
                win_ap(vT, 0, j, 0, CB),
            )
            for h in range(NH):
                kst = kst4[h][par]
                dma(
                    kst[D * h : D * h + D, :].rearrange("c (a b) -> c a b",
                                                        a=64),
                    win_ap(kT, 0, j, D * h, D),
                )
            # contiguous q-window tile (strided bf16 views are PE-fetch
            # bound): br0 gather via DMA (prefetchable), br1 via DVE
            qcw = padp.tile([128, TT], BF, tag="qcw")
            qcw3 = qcw[:].rearrange("c (a b) -> c a b", a=64)
            dma(qcw3[0:CB], win_ap(qT, 0, j, 0, CB))
            nc.vector.tensor_copy(qcw3[CB:128], win_ap(qT, 1, j, CB, CB))

            # LePE: zero-padded q window images (66 x 10), branches stacked;
            # 9 taps split DVE(5)/GPSIMD(4) with two accumulators
            pad = padp.tile([128, 660], BF, tag="pad")
            nc.gpsimd.memset(pad[:], 0.0)
            pad3 = pad[:].rearrange("c (h w) -> c h w", h=66)
            nc.gpsimd.tensor_copy(pad3[:, 1:65, 1:9], qcw3)
            lepe = padp.tile([128, TT], BF, tag="lepe")
            lepe3 = lepe[:].rearrange("c (h w) -> c h w", h=64)
            taps = [(a, b) for a in (-1, 0, 1) for b in (-1, 0, 1)]
            for idx, (a, b) in enumerate(taps):
                src = pad3[:, 1 + a : 65 + a, 1 + b : 9 + b]
                wc = wcomb[:, 3 * (a + 1) + (b + 1) : 3 * (a + 1) + (b + 2)]
                if idx == 0:
                    nc.vector.tensor_scalar_mul(lepe3, src, wc)
                else:
                    nc.vector.scalar_tensor_tensor(
                        lepe3, src, wc, lepe3, op0=OP.mult, op1=OP.add
                    )
            nc.gpsimd.tensor_copy(
                lepe8[CB:128, TT * j : TT * j + TT], lepe[CB:128, :]
            )
            v4 = build_v4(j, 0, vst)
            branch_attn(
                j, 0, qcw[0:CB, :], v4,
                lepe[0:CB, :].rearrange("c (h w) -> c h w", h=H),
            )

        # ---- phase B: branch-1 windows; proj lags 1 window, LN2 stats lag
        # 2 windows so their inputs are long-ready when the PE reaches them
        # (no head-of-line FIFO stalls) -----------------------------------
        xbs = {}
        xsqs = {}

        def proj_tile(t):
            sl = slice(TT * t, TT * t + TT)
            ap_ = ps_mm([128, TT], "ap_")
            mm(ap_[:], WpT[:], cat[:, sl], start=True, stop=True)
            nc.vector.scalar_tensor_tensor(
                xf2[:, sl], ap_[:], bprojc[:], xT[:, sl],
                op0=OP.add, op1=OP.add,
            )
            xb = stat.tile([128, TT], BF, tag="xb")
            nc.gpsimd.tensor_copy(xb[:], xf2[:, sl])
            xsq = stat.tile([128, TT], BF, tag="xsq")
            nc.gpsimd.tensor_mul(xsq[:], xb[:], xb[:])
            xbs[t] = xb
            xsqs[t] = xsq

        def stats_tile(t):
            sl = slice(TT * t, TT * t + TT)
            s1p = psum.tile([128, TT], F32, tag="tp", bufs=2, name="s1p")
            mm(s1p[:], ones_st[:], xbs[t][:], start=True, stop=True)
            s2p = psum.tile([128, TT], F32, tag="tp", bufs=2, name="s2p")
            mm(s2p[:], ones_st[:], xsqs[t][:], start=True, stop=True)
            msq = stat.tile([128, TT], F32, tag="msq")
            nc.scalar.activation(msq[:], s1p[:], AF.Square, scale=1.0 / C)
            var = ln3p.tile([128, TT], F32, tag=f"var_{t}", name=f"var_{t}")
            nc.vector.scalar_tensor_tensor(
                var[:], s2p[:], 1.0 / C, msq[:], op0=OP.mult, op1=OP.subtract
            )
            xc = ln3p.tile([128, TT], BF, tag=f"xc_{t}", name=f"xc_{t}")
            nc.vector.scalar_tensor_tensor(
                xc[:], s1p[:], -1.0 / C, xf2[:, sl], op0=OP.mult, op1=OP.add
            )
            var8.append(var)
            xc8.append(xc)

        for j in range(NWIN):
            par = j % 2
            vst = v4p.tile([128, TT], BF, tag="vst")
            dma(vst[CB:128, :], vT[CB:128, TT * j : TT * j + TT])
            for h in range(NH):
                kst = kst4[h][par]
                dma(
                    kst[CB + D * h : CB + D * h + D, :],
                    kT[CB + D * h : CB + D * h + D, TT * j : TT * j + TT],
                )
            qcw = padp.tile([128, TT], BF, tag="qcw")
            qcw3 = qcw[:].rearrange("c (a b) -> c a b", a=64)
            nc.vector.tensor_copy(qcw3[CB:128], win_ap(qT, 1, j, CB, CB))
            v4 = build_v4(j, 1, vst)
            branch_attn(
                j, 1, qcw[CB:128, :], v4,
                lepe8[CB:128, TT * j : TT * j + TT].rearrange(
                    "c (h w) -> c h w", h=H
                ),
            )
            if j >= 1:
                proj_tile(j - 1)
            if j >= 2:
                stats_tile(j - 2)
        proj_tile(NWIN - 1)
        stats_tile(NWIN - 2)
        stats_tile(NWIN - 1)

    # ---------------- tail: rsqrt pass (sqrt table) then MLP (gelu) --------
    hns = []
    for t in range(NT):
        sd = stat.tile([128, TT], F32, tag="sd")
        nc.scalar.activation(
            sd[:], var8[t][:], AF.Sqrt, bias=epscol[:], scale=1.0
        )
        r = stat.tile([128, TT], F32, tag="r")
        nc.vector.reciprocal_approx_fast(out=r[:], in_=sd[:])
        hn = ln3p.tile([128, TT], BF, tag=f"hn2_{t}", name=f"hn2_{t}")
        nc.vector.tensor_mul(hn[:], xc8[t][:], r[:])
        hns.append(hn)

    with tc.tile_pool(name="ps3b", bufs=1, space="PSUM") as ps3b:
        for t in range(NT):
            sl = slice(TT * t, TT * t + TT)
            hn = hns[t]
            gel = gelp.tile([128, 4 * TT], BF, tag="gel")
            for hh in range(4):
                hp = ps3b.tile([128, TT], F32, tag="hp", bufs=2, name="hp")
                mm(
                    hp[:],
                    W1gT[:, 128 * hh : 128 * hh + 128],
                    hn[:],
                    start=True, stop=True,
                )
                nc.scalar.activation(
                    gel[:, TT * hh : TT * hh + TT],
                    hp[:],
                    AF.Gelu,
                    bias=gbcols[:, hh : hh + 1],
                    scale=1.0,
                )
            o2 = ps_mm([128, TT], "o2")
            for hh in range(4):
                mm(
                    o2[:],
                    W2T[:, 128 * hh : 128 * hh + 128],
                    gel[:, TT * hh : TT * hh + TT],
                    start=(hh == 0), stop=(hh == 3),
                )
            ot = outp.tile([128, TT], F32, tag="ot")
            nc.vector.scalar_tensor_tensor(
                ot[:], o2[:], b2c[:], xf2[:, sl], op0=OP.add, op1=OP.add
            )
            dma(io["out"][:, sl], ot[:])


_NC_CACHE = {}


def build_nc():
    key = "nc"
    if key in _NC_CACHE:
        return _NC_CACHE[key]
    nc = bacc.Bacc("TRN2", target_bir_lowering=False, debug=False)
    io = {}
    for name in INPUT_NAMES:
        io[name] = nc.dram_tensor(
            name, INPUT_SHAPES[name], F32, kind="ExternalInput"
        ).ap()
    io["out"] = nc.dram_tensor("out", [C, L], F32, kind="ExternalOutput").ap()
    with tile.TileContext(nc) as tc:
        with ExitStack() as ctx:
            emit(ctx, tc, io)
    nc.compile()
    _NC_CACHE[key] = nc
    return nc


def make_in_maps(inputs):
    in_maps = []
    for b in range(B):
        m = {
            "x": np.ascontiguousarray(
                inputs["x"][b].reshape(C, L).astype(np.float32)
            ),
            "z": np.ascontiguousarray(inputs["z"][b].astype(np.float32)),
        }
        for name in INPUT_NAMES:
            if name in ("x", "z"):
                continue
            m[name] = np.ascontiguousarray(np.asarray(inputs[name], np.float32))
        in_maps.append(m)
    return in_maps


def kernel(**inputs):
    nc = build_nc()
    in_maps = make_in_maps(inputs)
    res = bass_utils.run_bass_kernel_spmd(nc, in_maps, list(range(B)))
    out = np.stack([res.results[b]["out"].reshape(C, H, W) for b in range(B)])
    return out.astype(np.float32)


if __name__ == "__main__":
    # CoreSim numerics check of core 0 against the reference (dev only).
    import sys

    sys.path.insert(0, "/root/problem")
    import reference

    from concourse.bass_interp import CoreSim

    # CoreSim has no Gelu; patch it (HW has a native erf-gelu table).
    import scipy.special
    from concourse import bass_interp

    _orig_act = bass_interp.InstructionExecutor.visit_InstActivation

    def _patched_act(self, instruction, *, reg_snapshot=None):
        if instruction.func == mybir.ActivationFunctionType.Gelu:
            instruction.func = mybir.ActivationFunctionType.Identity
            try:
                _orig_act(self, instruction, reg_snapshot=reg_snapshot)
            finally:
                instruction.func = mybir.ActivationFunctionType.Gelu
            ov = self.view_ap(
                instruction.outs[0],
                bass_interp.Direction.WRITE,
                instruction,
                reg_snapshot=reg_snapshot,
            )
            x = ov.astype(np.float64)
            ov[:] = (
                x * 0.5 * (1.0 + scipy.special.erf(x / np.sqrt(2.0)))
            ).astype(ov.dtype)
            return
        return _orig_act(self, instruction, reg_snapshot=reg_snapshot)

    bass_interp.InstructionExecutor.visit_InstActivation = _patched_act

    inputs = {k: np.asarray(v) for k, v in reference.setup_inputs().items()}
    expected = np.asarray(reference.reference(**inputs))

    nc = build_nc()
    print("built+compiled", flush=True)
    sim = CoreSim(nc, require_finite=True, require_nnan=True)
    m = make_in_maps(inputs)[0]
    for k, v in m.items():
        sim.tensor(k)[:] = v
    sim.simulate(check_with_hw=False)
    got = sim.tensor("out").reshape(C, H, W)
    exp0 = expected[0]
    err = np.abs(got - exp0)
    denom = np.abs(exp0).max()
    print("absmax err:", err.max(), "rel:", err.max() / denom)
    print(
        "rms rel:",
        np.sqrt(((got - exp0) ** 2).mean()) / np.sqrt((exp0**2).mean()),
    )


# revision 25
# speedup vs baseline: 1.1256x; 1.0948x over previous
"""Trainium2 Bass kernel for DegradationRectifyNet block (CSWin-style window
attention + LePE depthwise conv + code-conditioned LN/MLP).

Data-parallel over batch B=8 across 8 NeuronCores; one image per core.
On-chip everything is channel-major (C on partitions, tokens on free dim).
Compute dtype is bf16 (validated ~2.6e-3 max-rel vs the 2e-2 gate); PSUM
accumulation stays fp32.

Per-core pipeline:
  LN stats via all-ones PE matmul (partition reduction, broadcast-redundant)
  with Square/Sqrt on the scalar engine and a fast approximate reciprocal on
  DVE; LN affine + z-shift folded into transposed qkv weights. Window
  attention computes S^T (k on partitions) in 1-bank PSUM quarters (3-deep
  rotation) so exp of chunk c overlaps the matmul of chunk c+1; softmax
  denominators come from a ones-column appended to the v stationary; the
  per-query division is done in a transposed frame via bf16 PE transposes.
  LePE depthwise 3x3 runs as 9 block-diagonal PE matmuls over zero-padded
  window images (branch-1 windows stored transposed so both branches share
  geometry). Proj and the MLP are bf16 matmuls with biases/residuals fused
  into scalar_tensor_tensor epilogues; the MLP phase is split into a stats
  pass (sqrt table) and a gelu pass (gelu table) to avoid act-table thrash.

PE quadrant constraints (operand partition bases must be 32-aligned) are
handled by: (a) computing QK^T with 64-row contractions against k copies that
have the other heads zeroed (kst slabs, double-buffered across windows), and
(b) padding AV output stripes to 32 partitions with explicit zero columns in
the stationary.
"""

import numpy as np

import concourse.bass as bass
import concourse.bacc as bacc
import concourse.tile as tile
from concourse import mybir
from concourse import bass_utils
from concourse.masks import make_identity
from contextlib import ExitStack

F32 = mybir.dt.float32
BF = mybir.dt.bfloat16
AF = mybir.ActivationFunctionType
OP = mybir.AluOpType

B = 8
C = 128          # channels
H = W = 64
L = H * W        # 4096 tokens
SS = 8           # split size
CB = C // 2      # branch channels = 64
NH = 4           # heads per branch
D = CB // NH     # head dim = 16
HID = 4 * C      # 512
CHZ = 512
NT = 8           # token tiles
TT = 512         # tokens per tile
NWIN = 8         # windows per branch
EPS = 1e-5

INPUT_NAMES = [
    "x", "z", "ln1_g", "ln1_b", "ln2_g", "ln2_b", "Wz1", "Wz2", "Wqkv",
    "Wproj", "bproj", "lepe_w0", "lepe_b0", "lepe_w1", "lepe_b1",
    "W1", "b1", "W2", "b2",
]
INPUT_SHAPES = {
    "x": [C, L], "z": [CHZ],
    "ln1_g": [C], "ln1_b": [C], "ln2_g": [C], "ln2_b": [C],
    "Wz1": [C, CHZ], "Wz2": [C, CHZ], "Wqkv": [3 * C, C],
    "Wproj": [C, C], "bproj": [C],
    "lepe_w0": [CB, 1, 3, 3], "lepe_b0": [CB],
    "lepe_w1": [CB, 1, 3, 3], "lepe_b1": [CB],
    "W1": [HID, C], "b1": [HID], "W2": [C, HID], "b2": [C],
}


def emit(ctx: ExitStack, tc: tile.TileContext, io: dict):
    nc = tc.nc

    consts = ctx.enter_context(tc.tile_pool(name="consts", bufs=1))
    wpool = ctx.enter_context(tc.tile_pool(name="wpool", bufs=1))
    big = ctx.enter_context(tc.tile_pool(name="big", bufs=1))
    stat = ctx.enter_context(tc.tile_pool(name="stat", bufs=2))
    padp = ctx.enter_context(tc.tile_pool(name="padp", bufs=2))
    expp = ctx.enter_context(tc.tile_pool(name="expp", bufs=2))
    v4p = ctx.enter_context(tc.tile_pool(name="v4p", bufs=2))
    epip = ctx.enter_context(tc.tile_pool(name="epip", bufs=2))
    kstp = ctx.enter_context(tc.tile_pool(name="kstp", bufs=1))
    gelp = ctx.enter_context(tc.tile_pool(name="gelp", bufs=2))
    outp = ctx.enter_context(tc.tile_pool(name="outp", bufs=2))
    ln3p = ctx.enter_context(tc.tile_pool(name="ln3p", bufs=1))

    # long-lived PSUM: "mm" (2 banks) for f32 matmul outs, "tp" (2 banks)
    # for bf16 transpose outs. Phase-scoped pools below add <= 4 banks.
    psum = ctx.enter_context(tc.tile_pool(name="psum", bufs=1, space="PSUM"))

    def ps_mm(shape, name):
        return psum.tile(shape, F32, tag="mm", bufs=2, name=name)

    def ps_tp(shape, name):
        return psum.tile(shape, BF, tag="tp", bufs=2, name=name)

    dma = nc.sync.dma_start
    mm = nc.tensor.matmul

    # ---------------- constants + x load (phase-1 critical path first) ----
    xT = big.tile([128, L], F32, tag="xT")
    for t in range(NT):
        dma(xT[:, TT * t : TT * t + TT], io["x"][:, TT * t : TT * t + TT])
    ident = consts.tile([128, 128], F32, tag="ident")
    make_identity(nc, ident[:])
    ones_st = consts.tile([128, 128], BF, tag="ones_st")
    nc.gpsimd.memset(ones_st[:], 1.0)
    ones_stF = consts.tile([128, 128], F32, tag="ones_stF")
    nc.gpsimd.memset(ones_stF[:], 1.0)
    epscol = consts.tile([128, 1], F32, tag="epscol")
    nc.gpsimd.memset(epscol[:], EPS)

    def col(name):
        t = consts.tile([128, 1], F32, tag="col_" + name)
        dma(t[:], io[name].unsqueeze(1))
        return t

    g1c = col("ln1_g")
    bln1 = col("ln1_b")
    g2c = col("ln2_g")
    bln2 = col("ln2_b")
    bprojc = col("bproj")
    b2c = col("b2")

    b1cols = consts.tile([128, 4], F32, tag="b1cols")
    for h in range(4):
        dma(b1cols[:, h : h + 1], io["b1"][128 * h : 128 * h + 128].unsqueeze(1))
    zcols = consts.tile([128, 4], F32, tag="zcols")
    for k in range(4):
        dma(zcols[:, k : k + 1], io["z"][128 * k : 128 * k + 128].unsqueeze(1))

    # ---------------- weight transposes ----------------
    def load_transposed(dst_ap, src_ap, scale_col=None, copy_dst=None):
        wt = wpool.tile([128, 128], F32, tag="wtmp", bufs=4)
        dma(wt[:], src_ap)
        pt = ps_mm([128, 128], "ptw")
        nc.tensor.transpose(pt[:], wt[:], ident[:])
        if scale_col is not None:
            nc.vector.tensor_scalar_mul(dst_ap, pt[:], scale_col)
        else:
            nc.vector.tensor_copy(dst_ap, pt[:])
        if copy_dst is not None:
            nc.vector.tensor_copy(copy_dst, pt[:])

    WqkvT = wpool.tile([128, 3 * C], F32, tag="WqkvT")
    WgT = wpool.tile([128, 3 * C], BF, tag="WgT")
    for j in range(3):
        load_transposed(
            WgT[:, 128 * j : 128 * j + 128],
            io["Wqkv"][128 * j : 128 * j + 128, :],
            scale_col=g1c[:],
            copy_dst=WqkvT[:, 128 * j : 128 * j + 128],
        )

    Wz1T = wpool.tile([128, CHZ], F32, tag="Wz1T")
    for k in range(4):
        load_transposed(
            Wz1T[:, 128 * k : 128 * k + 128], io["Wz1"][:, 128 * k : 128 * k + 128]
        )

    # ---------------- z-conditioned bias columns ----------------
    def zbias(WzT, blnc, tag):
        zp = ps_mm([128, 1], "zp")
        for k in range(4):
            mm(
                zp[:], WzT[:, 128 * k : 128 * k + 128], zcols[:, k : k + 1],
                start=(k == 0), stop=(k == 3),
            )
        bz = consts.tile([128, 1], F32, tag=tag)
        nc.vector.tensor_add(bz[:], zp[:], blnc[:])
        return bz

    bz1 = zbias(Wz1T, bln1, "bz1")

    w0cols = consts.tile([128, 3], F32, tag="w0cols")
    for j in range(3):
        wp = ps_mm([128, 1], "wp")
        mm(wp[:], WqkvT[:, 128 * j : 128 * j + 128], bz1[:], start=True, stop=True)
        nc.vector.tensor_copy(w0cols[:, j : j + 1], wp[:])

    # ---------------- big activation buffers ----------------
    qT = big.tile([128, L], BF, tag="qT")
    kT = big.tile([128, L], BF, tag="kT")
    vT = big.tile([128, L], BF, tag="vT")
    kst4 = [
        [
            kstp.tile([128, TT], BF, tag=f"kst{i}_{p}", name=f"kst{i}_{p}")
            for p in range(2)
        ]
        for i in range(NH)
    ]
    cat = big.tile([128, L], BF, tag="cat")
    xf2 = big.tile([128, L], F32, tag="xf2")

    # ---------------- layernorm over channel (partition) dim ----------------
    # returns hn = (x - mean)/sqrt(var+eps) in bf16; keep_rxc also returns
    # (r, xc) tiles for the split phase-3 pipeline.
    def layernorm_tile(src, t, half_psum):
        sl = slice(TT * t, TT * t + TT)
        xt = src[:, sl]
        xsq = stat.tile([128, TT], BF, tag="xsq")
        nc.scalar.activation(xsq[:], xt, AF.Square)
        s12 = half_psum()
        s1 = s12[:, 0:TT]
        s2 = s12[:, TT : 2 * TT]
        mm(s1, ones_stF[:], xt, start=True, stop=True)
        mm(s2, ones_st[:], xsq[:], start=True, stop=True)
        msq = stat.tile([128, TT], F32, tag="msq")
        nc.scalar.activation(msq[:], s1, AF.Square, scale=1.0 / C)
        var = stat.tile([128, TT], F32, tag="var")
        nc.vector.scalar_tensor_tensor(
            var[:], s2, 1.0 / C, msq[:], op0=OP.mult, op1=OP.subtract
        )
        sd = stat.tile([128, TT], F32, tag="sd")
        nc.scalar.activation(sd[:], var[:], AF.Sqrt, bias=epscol[:], scale=1.0)
        r = stat.tile([128, TT], F32, tag="r")
        nc.vector.reciprocal_approx_fast(out=r[:], in_=sd[:])
        xc = stat.tile([128, TT], BF, tag="xc")
        nc.vector.scalar_tensor_tensor(
            xc[:], s1, -1.0 / C, xt, op0=OP.mult, op1=OP.add
        )
        return r, xc

    # ---------------- phase 1: LN1 + qkv ----------------
    with tc.tile_pool(name="ps1", bufs=1, space="PSUM") as ps1:
        for t in range(NT):
            sl = slice(TT * t, TT * t + TT)
            r, xc = layernorm_tile(
                xT, t,
                lambda: ps1.tile([128, 2 * TT], F32, tag="half", bufs=2,
                                 name="s12"),
            )
            hn = stat.tile([128, TT], BF, tag="hn")
            nc.vector.tensor_mul(hn[:], xc[:], r[:])
            for j, dstT in enumerate((qT, kT, vT)):
                qp = ps_mm([128, TT], "qp")
                mm(
                    qp[:], WgT[:, 128 * j : 128 * j + 128], hn[:],
                    start=True, stop=True,
                )
                if j == 2:
                    nc.vector.tensor_scalar_add(
                        dstT[:, sl], qp[:], w0cols[:, j : j + 1]
                    )
                else:
                    nc.scalar.activation(
                        dstT[:, sl], qp[:], AF.Identity,
                        bias=w0cols[:, j : j + 1], scale=1.0,
                    )

    # ---------------- attention constants (after phase-1 emission) ----------
    identB = consts.tile([128, 128], BF, tag="identB")
    nc.vector.tensor_copy(identB[:], ident[:])
    # 64x64 identity living at partitions 64:128 (for branch-1 v transposes)
    ident2f = consts.tile([128, 64], F32, tag="ident2f")
    nc.gpsimd.memset(ident2f[:], 0.0)
    nc.gpsimd.affine_select(
        out=ident2f[:], in_=ident2f[:], compare_op=OP.not_equal, fill=1.0,
        base=-64, pattern=[[-1, 64]], channel_multiplier=1,
    )
    ident2 = consts.tile([128, 64], BF, tag="ident2")
    nc.vector.tensor_copy(ident2[:], ident2f[:])
    lepebc = consts.tile([128, 1], F32, tag="lepebc")
    dma(lepebc[0:CB, :], io["lepe_b0"].unsqueeze(1))
    dma(lepebc[CB:128, :], io["lepe_b1"].unsqueeze(1))

    # conv tap weights: rows 0:64 branch-0 taps (a,b); rows 64:128 branch-1
    # taps transposed (branch-1 window images are stored transposed)
    wcomb = consts.tile([128, 9], F32, tag="wcomb")
    dma(wcomb[0:CB, :], io["lepe_w0"].rearrange("c o a b -> c (o a b)"))
    wstage = consts.tile([64, 9], F32, tag="wstage")
    dma(wstage[:], io["lepe_w1"].rearrange("c o a b -> c (o a b)"))
    nc.gpsimd.tensor_copy(
        wcomb[CB:128, :].rearrange("c (b a) -> c b a", b=3),
        wstage[:].rearrange("c (a b) -> c a b", a=3).transpose([0, 2, 1]),
    )
    # v4 template: 32-wide head slots, col 16 = 1.0 (denominator column)
    v4t = consts.tile([128, TT], BF, tag="v4t")
    nc.gpsimd.memset(v4t[:], 0.0)
    nc.vector.memset(
        v4t[:].rearrange("p (s w) -> p s w", s=16)[:, :, 16:17], 1.0
    )
    # QK^T stationary slabs: slab (h, parity) keeps only head h's 16 rows
    # live per branch (rows 64*br+16*h), rest stay zero, so the 64-row
    # contraction reads start at partition 0/64. Two parity buffers let
    # window j+1's DMAs land while j is computing.
    for i in range(NH):
        for p in range(2):
            nc.gpsimd.memset(kst4[i][p][:], 0.0)

    # ---------------- remaining weight setup (overlaps attention start) ----
    Wz2T = wpool.tile([128, CHZ], F32, tag="Wz2T")
    for k in range(4):
        load_transposed(
            Wz2T[:, 128 * k : 128 * k + 128], io["Wz2"][:, 128 * k : 128 * k + 128]
        )
    bz2 = zbias(Wz2T, bln2, "bz2")

    W1T = wpool.tile([128, HID], F32, tag="W1T")
    W1gT = wpool.tile([128, HID], BF, tag="W1gT")
    for h in range(4):
        load_transposed(
            W1gT[:, 128 * h : 128 * h + 128],
            io["W1"][128 * h : 128 * h + 128, :],
            scale_col=g2c[:],
            copy_dst=W1T[:, 128 * h : 128 * h + 128],
        )
    W2T = wpool.tile([128, HID], BF, tag="W2T")
    for h in range(4):
        load_transposed(
            W2T[:, 128 * h : 128 * h + 128], io["W2"][:, 128 * h : 128 * h + 128]
        )
    WpT = wpool.tile([128, C], BF, tag="WpT")
    load_transposed(WpT[:], io["Wproj"][:, :])

    gbcols = consts.tile([128, 4], F32, tag="gbcols")
    for h in range(4):
        wp = ps_mm([128, 1], "wp")
        mm(wp[:], W1T[:, 128 * h : 128 * h + 128], bz2[:], start=True, stop=True)
        nc.vector.tensor_add(gbcols[:, h : h + 1], wp[:], b1cols[:, h : h + 1])

    # window access patterns -------------------------------------------------
    # branch 0: vertical strip window j = cols [8j,8j+8); raster (h, w)
    # branch 1: horizontal strip window j, stored transposed; raster (w, h)
    def win_ap(src, br, j, p0, psz):
        a3 = src[p0 : p0 + psz, :].rearrange("c (h w) -> c h w", h=H)
        if br == 0:
            return a3[:, :, SS * j : SS * j + SS]
        return a3[:, SS * j : SS * j + SS, :].transpose([0, 2, 1])

    # ---------------- attention + lepe, one window pair per j --------------
    # Unified per-window processing (both branches) keeps every engine's
    # per-window load balanced and the PE streak long enough to stay at the
    # ramped p-state; splitting branches into phases measured slower.
    with tc.tile_pool(name="ps2", bufs=1, space="PSUM") as ps2:

        def build_v4(j, br, vst):
            # v': token-major v (via PE transpose), 32-wide head slots:
            # cols [0:16) v, col 16 ones (denominator), cols [17:32) zero
            p0 = CB * br
            vps = ps_tp([128, 256], "vps")
            idv = identB[0:CB, 0:CB] if br == 0 else ident2[CB:128, :]
            for c in range(4):
                mm(
                    vps[:, 64 * c : 64 * c + 64],
                    vst[p0 : p0 + CB, 128 * c : 128 * c + 128],
                    idv,
                    is_transpose=True,
                    start=(c == 0), stop=(c == 3),
                )
            v4 = v4p.tile([128, TT], BF, tag="v4")
            nc.vector.tensor_copy(v4[:], v4t[:])
            v4v = v4[:].rearrange("p (c h s) -> p c h s", c=4, h=4)
            nc.vector.tensor_copy(
                v4v[:, :, :, 0:16],
                vps[:].rearrange("p (c h d) -> p c h d", c=4, h=4),
            )
            return v4

        def branch_attn(j, br, qw, v4, lepe_view):
            par = j % 2
            p0 = CB * br
            A = epip.tile([128, TT], BF, tag="A")
            for h in range(NH):
                kst = kst4[h][par]
                es = expp.tile([128, 4 * TT], BF, tag="es")
                # 2-bank score halves, 2-deep rotation: exp of half a
                # overlaps the matmuls of half a+1
                for a in range(2):
                    sp = ps2.tile([128, 2 * TT], F32, tag="sp", bufs=2,
                                  name="sp")
                    for cc in range(2):
                        c = 2 * a + cc
                        mm(
                            sp[:, TT * cc : TT * cc + TT],
                            kst[p0 : p0 + CB, 128 * c : 128 * c + 128],
                            qw,
                            start=True, stop=True,
                        )
                    nc.scalar.activation(
                        es[:, 2 * TT * a : 2 * TT * a + 2 * TT], sp[:],
                        AF.Exp, scale=float(D) ** -0.5,
                    )
                avh = ps_mm([32, TT], "avh")
                for c in range(4):
                    mm(
                        avh[:],
                        v4[:, 128 * c + 32 * h : 128 * c + 32 * h + 32],
                        es[:, TT * c : TT * c + TT],
                        start=(c == 0), stop=(c == 3),
                    )
                nc.vector.tensor_copy(A[32 * h : 32 * h + 32, :], avh[:])

            # epilogue: transpose -> divide by denominators -> transpose
            Tp = ps_tp([128, TT], "Tp")
            for c in range(4):
                mm(
                    Tp[:, 128 * c : 128 * c + 128],
                    A[:, 128 * c : 128 * c + 128],
                    identB[:],
                    is_transpose=True,
                    start=(c == 0), stop=(c == 3),
                )
            Tv = Tp.rearrange("p (c h s) -> p c h s", c=4, h=4)
            dcol = epip.tile([128, 16], F32, tag="dcol")
            nc.vector.tensor_copy(
                dcol[:].rearrange("p (c h) -> p c h", c=4), Tv[:, :, :, 16]
            )
            R = epip.tile([128, 16], F32, tag="R")
            nc.vector.reciprocal_approx_fast(out=R[:], in_=dcol[:])
            Rv = R[:].rearrange("p (c h) -> p c h", c=4)
            E = epip.tile([128, 256], BF, tag="E")
            Ev = E[:].rearrange("p (c h d) -> p c h d", c=4, h=4)
            nc.vector.tensor_mul(
                Ev[:, :, :, :],
                Tv[:, :, :, 0:16],
                Rv[:, :, :].unsqueeze(3).broadcast_to((128, 4, 4, 16)),
            )
            Ot = ps_tp([CB, TT], "Ot")
            for c in range(4):
                mm(
                    Ot[:, 128 * c : 128 * c + 128],
                    E[:, 64 * c : 64 * c + 64],
                    identB[:],
                    is_transpose=True,
                    start=(c == 0), stop=(c == 3),
                )
            # un-window: attention + lepe(+bias) into cat rows [64*br,+64)
            nc.vector.scalar_tensor_tensor(
                win_ap(cat, br, j, p0, CB),
                Ot[:].rearrange("c (h w) -> c h w", h=H),
                lepebc[p0 : p0 + CB, :],
                lepe_view,
                op0=OP.add, op1=OP.add,
            )

        for j in range(NWIN):
            par = j % 2
            vst = v4p.tile([128, TT], BF, tag="vst")
            dma(
                vst[0:CB, :].rearrange("c (a b) -> c a b", a=64),
                win_ap(vT, 0, j, 0, CB),
            )
            dma(vst[CB:128, :], vT[CB:128, TT * j : TT * j + TT])
            for h in range(NH):
                kst = kst4[h][par]
                dma(
                    kst[D * h : D * h + D, :].rearrange("c (a b) -> c a b",
                                                        a=64),
                    win_ap(kT, 0, j, D * h, D),
                )
                dma(
                    kst[CB + D * h : CB + D * h + D, :],
                    kT[CB + D * h : CB + D * h + D, TT * j : TT * j + TT],
                )
            # contiguous q-window tile (strided bf16 views are PE-fetch
            # bound): br0 gather via DMA (prefetchable), br1 via DVE
            qcw = padp.tile([128, TT], BF, tag="qcw")
            qcw3 = qcw[:].rearrange("c (a b) -> c a b", a=64)
            dma(qcw3[0:CB], win_ap(qT, 0, j, 0, CB))
            nc.vector.tensor_copy(qcw3[CB:128], win_ap(qT, 1, j, CB, CB))

            # LePE: zero-padded q window images (66 x 10), branches stacked;
            # 9 tap accumulations on DVE (keeps the PE free for QK/AV)
            pad = padp.tile([128, 660], BF, tag="pad")
            nc.gpsimd.memset(pad[:], 0.0)
            pad3 = pad[:].rearrange("c (h w) -> c h w", h=66)
            nc.gpsimd.tensor_copy(pad3[:, 1:65, 1:9], qcw3)
            lepe = padp.tile([128, TT], BF, tag="lepe")
            lepe3 = lepe[:].rearrange("c (h w) -> c h w", h=64)
            taps = [(a, b) for a in (-1, 0, 1) for b in (-1, 0, 1)]
            for idx, (a, b) in enumerate(taps):
                src = pad3[:, 1 + a : 65 + a, 1 + b : 9 + b]
                wc = wcomb[:, 3 * (a + 1) + (b + 1) : 3 * (a + 1) + (b + 2)]
                if idx == 0:
                    nc.vector.tensor_scalar_mul(lepe3, src, wc)
                else:
                    nc.vector.scalar_tensor_tensor(
                        lepe3, src, wc, lepe3, op0=OP.mult, op1=OP.add
                    )

            for br in range(2):
                p0 = CB * br
                v4 = build_v4(j, br, vst)
                branch_attn(
                    j, br, qcw[p0 : p0 + CB, :], v4,
                    lepe[p0 : p0 + CB, :].rearrange("c (h w) -> c h w", h=H),
                )

    # ---------------- proj + residual ----------------
    for t in range(NT):
        sl = slice(TT * t, TT * t + TT)
        ap_ = ps_mm([128, TT], "ap_")
        mm(ap_[:], WpT[:], cat[:, sl], start=True, stop=True)
        nc.vector.scalar_tensor_tensor(
            xf2[:, sl], ap_[:], bprojc[:], xT[:, sl], op0=OP.add, op1=OP.add
        )

    # ---------------- LN2 stats pass (sqrt table), then MLP (gelu table) ----
    hns = []
    with tc.tile_pool(name="ps3a", bufs=1, space="PSUM") as ps3a:
        for t in range(NT):
            r, xc = layernorm_tile(
                xf2, t,
                lambda: ps3a.tile([128, 2 * TT], F32, tag="half", bufs=2,
                                  name="s12b"),
            )
            hn = ln3p.tile([128, TT], BF, tag=f"hn2_{t}", name=f"hn2_{t}")
            nc.vector.tensor_mul(hn[:], xc[:], r[:])
            hns.append(hn)

    with tc.tile_pool(name="ps3b", bufs=1, space="PSUM") as ps3b:
        for t in range(NT):
            sl = slice(TT * t, TT * t + TT)
            hn = hns[t]
            gel = gelp.tile([128, 4 * TT], BF, tag="gel")
            for hh in range(4):
                hp = ps3b.tile([128, TT], F32, tag="hp", bufs=4, name="hp")
                mm(
                    hp[:],
                    W1gT[:, 128 * hh : 128 * hh + 128],
                    hn[:],
                    start=True, stop=True,
                )
                nc.scalar.activation(
                    gel[:, TT * hh : TT * hh + TT],
                    hp[:],
                    AF.Gelu,
                    bias=gbcols[:, hh : hh + 1],
                    scale=1.0,
                )
            o2 = ps_mm([128, TT], "o2")
            for hh in range(4):
                mm(
                    o2[:],
                    W2T[:, 128 * hh : 128 * hh + 128],
                    gel[:, TT * hh : TT * hh + TT],
                    start=(hh == 0), stop=(hh == 3),
                )
            ot = outp.tile([128, TT], F32, tag="ot")
            nc.vector.scalar_tensor_tensor(
                ot[:], o2[:], b2c[:], xf2[:, sl], op0=OP.add, op1=OP.add
            )
            dma(io["out"][:, sl], ot[:])


_NC_CACHE = {}


def build_nc():
    key = "nc"
    if key in _NC_CACHE:
        return _NC_CACHE[key]
    nc = bacc.Bacc("TRN2", target_bir_lowering=False, debug=False)
    io = {}
    for name in INPUT_NAMES:
        io[name] = nc.dram_tensor(
            name, INPUT_SHAPES[name], F32, kind="ExternalInput"
        ).ap()
    io["out"] = nc.dram_tensor("out", [C, L], F32, kind="ExternalOutput").ap()
    with tile.TileContext(nc) as tc:
        with ExitStack() as ctx:
            emit(ctx, tc, io)
    nc.compile()
    _NC_CACHE[key] = nc
    return nc


def make_in_maps(inputs):
    in_maps = []
    for b in range(B):
        m = {
            "x": np.ascontiguousarray(
                inputs["x"][b].reshape(C, L).astype(np.float32)
            ),
            "z": np.ascontiguousarray(inputs["z"][b].astype(np.float32)),
        }
        for name in INPUT_NAMES:
            if name in ("x", "z"):
                continue
            m[name] = np.ascontiguousarray(np.asarray(inputs[name], np.float32))
        in_maps.append(m)
    return in_maps


def kernel(**inputs):
    nc = build_nc()
    in_maps = make_in_maps(inputs)
    res = bass_utils.run_bass_kernel_spmd(nc, in_maps, list(range(B)))
    out = np.stack([res.results[b]["out"].reshape(C, H, W) for b in range(B)])
    return out.astype(np.float32)


if __name__ == "__main__":
    # CoreSim numerics check of core 0 against the reference (dev only).
    import sys

    sys.path.insert(0, "/root/problem")
    import reference

    from concourse.bass_interp import CoreSim

    # CoreSim has no Gelu; patch it (HW has a native erf-gelu table).
    import scipy.special
    from concourse import bass_interp

    _orig_act = bass_interp.InstructionExecutor.visit_InstActivation

    def _patched_act(self, instruction, *, reg_snapshot=None):
        if instruction.func == mybir.ActivationFunctionType.Gelu:
            instruction.func = mybir.ActivationFunctionType.Identity
            try:
                _orig_act(self, instruction, reg_snapshot=reg_snapshot)
            finally:
                instruction.func = mybir.ActivationFunctionType.Gelu
            ov = self.view_ap(
                instruction.outs[0],
                bass_interp.Direction.WRITE,
                instruction,
                reg_snapshot=reg_snapshot,
            )
            x = ov.astype(np.float64)
            ov[:] = (
                x * 0.5 * (1.0 + scipy.special.erf(x / np.sqrt(2.0)))
            ).astype(ov.dtype)
            return
        return _orig_act(self, instruction, reg_snapshot=reg_snapshot)

    bass_interp.InstructionExecutor.visit_InstActivation = _patched_act

    inputs = {k: np.asarray(v) for k, v in reference.setup_inputs().items()}
    expected = np.asarray(reference.reference(**inputs))

    nc = build_nc()
    print("built+compiled", flush=True)
    sim = CoreSim(nc, require_finite=True, require_nnan=True)
    m = make_in_maps(inputs)[0]
    for k, v in m.items():
        sim.tensor(k)[:] = v
    sim.simulate(check_with_hw=False)
    got = sim.tensor("out").reshape(C, H, W)
    exp0 = expected[0]
    err = np.abs(got - exp0)
    denom = np.abs(exp0).max()
    print("absmax err:", err.max(), "rel:", err.max() / denom)
    print(
        "rms rel:",
        np.sqrt(((got - exp0) ** 2).mean()) / np.sqrt((exp0**2).mean()),
    )
